# revision 9
# baseline (speedup 1.0000x reference)
"""Causal self-attention (GQA + RoPE + qk gains) on 8 Trainium2 cores.

Sharding: tensor-parallel over the 4 KV head groups (cores c%4) x
data-parallel over batch pairs (cores c//4). Each core computes its 4 query
heads / 1 kv head for 2 batches and a partial output projection; the host
sums the 4 TP partials per batch group.

Device kernel layout notes:
  - x is shipped pre-transposed (C-major) in bf16 so every projection matmul
    contracts over C with no on-device transposes.
  - Attention computes S^T = K @ Q^T blocks so softmax's exp writes P^T
    directly PSUM->SBUF (ScalarE) with no PE transposes of P; row sums come
    from a ones-matmul that accumulates alongside AV.
  - exp needs no max subtraction: logits are ~N(0,1) for this problem's
    input distribution (|s| < ~7), well within fp32/bf16 exp range.
"""
import numpy as np
import ml_dtypes

import concourse.bass as bass
import concourse.mybir as mybir
import concourse.tile as tile
from concourse.masks import make_identity
from concourse.bass_utils import run_bass_kernel_spmd

B, T, C = 4, 2048, 2048
H, KV, D = 16, 4, 128
HL = H // KV          # local q heads per core
ROPE_BASE = 10000.0
NCORES = 8
KC = C // 128         # contraction chunks for projections
NT = T // 512         # 512-wide token tiles
NB = 2                # local batches per core

BF16 = mybir.dt.bfloat16
F32 = mybir.dt.float32
AF = mybir.ActivationFunctionType
ALU = mybir.AluOpType


class _TileContext(tile.TileContext):
    """This walrus build rejects instructions carrying more than 2 sync
    waits. After Tile finishes scheduling, hoist excess waits onto
    standalone same-engine NoOps placed just before the affected
    instruction (semantically identical: the engine stalls on the nops
    first)."""

    _MAXW = 1
    split_waits = True    # CoreSim can't model the injected nops; HW needs them

    def __exit__(self, exc_type, exc_val, exc_tb):
        r = super().__exit__(exc_type, exc_val, exc_tb)
        if exc_type is None and self.split_waits:
            nid = 0
            for fn in self.nc.m.functions:
                for bb in fn.blocks:
                    out = []
                    changed = False
                    for inst in bb.instructions:
                        si = inst.sync_info
                        waits = (list(si.on_wait)
                                 if si is not None and si.on_wait else [])
                        if len(waits) > self._MAXW:
                            changed = True
                            keep = waits[-self._MAXW:]
                            excess = waits[:-self._MAXW]
                            while excess:
                                chunk = excess[:self._MAXW]
                                excess = excess[self._MAXW:]
                                nop = mybir.InstNoOp(
                                    name=f"waitsplit-{nid}", ins=[], outs=[])
                                nid += 1
                                nop.engine = inst.engine
                                nop.sync_info = mybir.SyncInfo(
                                    on_wait=chunk, on_update=[])
                                out.append(nop)
                            si.on_wait = keep
                        out.append(inst)
                    if changed:
                        bb.instructions = out
        return r


def build_nc(reps: int = 1, _ablate: frozenset = frozenset(),
             split_waits: bool = True,
             pst_bufs: int = 4, ppt_bufs: int = 4, po_bufs: int = 2,
             hgroup: int = 1) -> bass.Bass:
    nc = bass.Bass("TRN2", target_bir_lowering=False, debug=False,
                   num_devices=NCORES)

    xt_in = nc.dram_tensor("xt", [NB, C, T], BF16, kind="ExternalInput")
    wqt_in = nc.dram_tensor("wqt", [C, HL * D], BF16, kind="ExternalInput")
    wkt_in = nc.dram_tensor("wkt", [C, D], BF16, kind="ExternalInput")
    wvt_in = nc.dram_tensor("wvt", [C, D], BF16, kind="ExternalInput")
    wot_in = nc.dram_tensor("wot", [HL * D, C], BF16, kind="ExternalInput")
    cosf_in = nc.dram_tensor("cosf", [D, T], F32, kind="ExternalInput")
    sinf_in = nc.dram_tensor("sinf", [D, T], F32, kind="ExternalInput")
    masks_in = nc.dram_tensor("masks", [128, HL, 512], BF16,
                              kind="ExternalInput")
    gsc_in = nc.dram_tensor("gsc", [128, HL], F32, kind="ExternalInput")
    out_dram = nc.dram_tensor("out", [NB, T, C], F32, kind="ExternalOutput")

    _TileContext.split_waits = split_waits
    with _TileContext(nc, num_cores=NCORES) as tc:
        with (
            tc.tile_pool(name="weights", bufs=1) as wpool,
            tc.tile_pool(name="xstream", bufs=2) as xpool,
            tc.tile_pool(name="acts", bufs=1) as apool,
            tc.tile_pool(name="ppt", bufs=ppt_bufs) as pptpool,
            tc.tile_pool(name="rtmp", bufs=2) as rpool,
            tc.tile_pool(name="outsb", bufs=2) as opool,
        ):
            # ---- weights / constants into SBUF ----
            wqt_s = wpool.tile([128, KC, HL * D], BF16)
            nc.sync.dma_start(
                wqt_s[:], wqt_in.rearrange("(kc p) m -> p kc m", p=128))
            wkt_s = wpool.tile([128, KC, D], BF16)
            nc.sync.dma_start(
                wkt_s[:], wkt_in.rearrange("(kc p) m -> p kc m", p=128))
            wvt_s = wpool.tile([128, KC, D], BF16)
            nc.sync.dma_start(
                wvt_s[:], wvt_in.rearrange("(kc p) m -> p kc m", p=128))
            wot_s = wpool.tile([128, HL, C], BF16)
            nc.sync.dma_start(
                wot_s[:], wot_in.rearrange("(kh p) n -> p kh n", p=128))
            cosf = wpool.tile([D, T], F32)
            nc.sync.dma_start(cosf[:], cosf_in[:])
            sinf = wpool.tile([D, T], F32)
            nc.sync.dma_start(sinf[:], sinf_in[:])
            masks_s = wpool.tile([128, HL, 512], BF16)
            nc.sync.dma_start(masks_s[:], masks_in[:])
            gb = wpool.tile([128, HL], F32)
            nc.sync.dma_start(gb[:], gsc_in[:])
            ones_s = wpool.tile([128, 128], BF16)
            nc.vector.memset(ones_s[:], 1.0)
            ident = wpool.tile([128, 128], BF16)
            make_identity(nc, ident[:])

            # activations (single-batch residency, reused across b)
            qT_s = apool.tile([128, HL, T], BF16)   # [d, h, tq]
            kT_s = apool.tile([128, T], BF16)       # [d, tk]
            V_s = apool.tile([128, T // 128, D], BF16)  # [tk%128, ck, d]
            yT_s = apool.tile([128, HL, T], BF16)   # [d, h, tq]

            def rope_store(psrc, dst, ncos, nsin):
                # dst = psrc*cosF + swap(psrc)*sinF   (sign baked into sinF)
                tsw = rpool.tile([128, 512], F32, tag="tswap")
                nc.scalar.copy(tsw[0:64, :], psrc[64:128, :])
                nc.scalar.copy(tsw[64:128, :], psrc[0:64, :])
                tco = rpool.tile([128, 512], F32, tag="tcos")
                nc.vector.tensor_tensor(tco[:], psrc[:], ncos, ALU.mult)
                nc.vector.tensor_tensor(tsw[:], tsw[:], nsin, ALU.mult)
                nc.vector.tensor_tensor(dst, tco[:], tsw[:], ALU.add)

            for _ in range(reps):
                for b in range(NB):
                    # ---------- phase A: q/k/v projections + rope ----------
                    if "A" in _ablate:
                        continue
                    with tc.tile_pool(name=f"psA{b}", bufs=1,
                                      space="PSUM") as psA:
                        for nt in range(NT):
                            xt_s = xpool.tile([128, KC, 512], BF16)
                            nc.sync.dma_start(
                                xt_s[:],
                                xt_in[b, :, nt * 512:(nt + 1) * 512]
                                .rearrange("(kc p) t -> p kc t", p=128))
                            pqs = [psA.tile([128, 512], F32, tag=f"pq{h}",
                                            name=f"pq{h}")
                                   for h in range(HL)]
                            pk = psA.tile([128, 512], F32, tag="pk")
                            pv = psA.tile([128, 512], F32, tag="pv")
                            for kc in range(KC):
                                st, sp = kc == 0, kc == KC - 1
                                for h in range(HL):
                                    nc.tensor.matmul(
                                        pqs[h][:],
                                        wqt_s[:, kc, h * 128:(h + 1) * 128],
                                        xt_s[:, kc, :], start=st, stop=sp)
                                nc.tensor.matmul(pk[:], wkt_s[:, kc, :],
                                                 xt_s[:, kc, :],
                                                 start=st, stop=sp)
                                nc.tensor.matmul(pv[:], wvt_s[:, kc, :],
                                                 xt_s[:, kc, :],
                                                 start=st, stop=sp)
                            ncos = cosf[:, nt * 512:(nt + 1) * 512]
                            nsin = sinf[:, nt * 512:(nt + 1) * 512]
                            for h in range(HL):
                                rope_store(pqs[h],
                                           qT_s[:, h, nt * 512:(nt + 1) * 512],
                                           ncos, nsin)
                            rope_store(pk, kT_s[:, nt * 512:(nt + 1) * 512],
                                       ncos, nsin)
                            vsb = rpool.tile([128, 512], BF16, tag="vsb")
                            nc.vector.tensor_copy(vsb[:], pv[:])
                            pvt = psA.tile([128, 512], BF16, tag="pvt")
                            for j in range(4):
                                nc.tensor.transpose(
                                    pvt[:, j * 128:(j + 1) * 128],
                                    vsb[:, j * 128:(j + 1) * 128], ident[:])
                            nc.vector.tensor_copy(
                                V_s[:, nt * 4:nt * 4 + 4, :],
                                pvt[:].rearrange("p (j d) -> p j d", j=4))

                    # ---------- phase B: causal attention ----------
                    if "B" in _ablate:
                        continue
                    with tc.tile_pool(name=f"psB{b}", bufs=po_bufs,
                                      space="PSUM") as psB:
                        for h0 in range(0, HL, hgroup):
                            hs = list(range(h0, min(h0 + hgroup, HL)))
                            for jq in range(NT):
                                nck = 4 * (jq + 1)
                                pos = {h: psB.tile([128, 512], F32,
                                                   tag=f"po{h - h0}",
                                                   name=f"po{h}")
                                       for h in hs}
                                psss = {h: psB.tile([128, 512], F32,
                                                    tag=f"pss{h - h0}",
                                                    name=f"pss{h}")
                                        for h in hs}
                                for ck in range(nck):
                                    for h in hs:
                                        pst = psB.tile([128, 512], F32,
                                                       tag="pst",
                                                       name="pst",
                                                       bufs=pst_bufs)
                                        nc.tensor.matmul(
                                            pst[:],
                                            kT_s[:, ck * 128:(ck + 1) * 128],
                                            qT_s[:, h,
                                                 jq * 512:(jq + 1) * 512],
                                            start=True, stop=True)
                                        ppt = pptpool.tile([128, 512], BF16)
                                        nc.scalar.activation(
                                            ppt[:], pst[:], AF.Exp,
                                            scale=gb[:, h:h + 1])
                                        r = ck - 4 * jq
                                        if r >= 0 and "mask" not in _ablate:
                                            nc.vector.tensor_tensor(
                                                ppt[:], ppt[:],
                                                masks_s[:, r, :], ALU.mult)
                                        nc.tensor.matmul(
                                            pos[h][:], V_s[:, ck, :], ppt[:],
                                            start=(ck == 0),
                                            stop=(ck == nck - 1))
                                        if "sum" not in _ablate:
                                            nc.tensor.matmul(
                                                psss[h][:], ones_s[:], ppt[:],
                                                start=(ck == 0),
                                                stop=(ck == nck - 1))
                                for h in hs:
                                    rec = rpool.tile([128, 512], F32,
                                                     tag="rec", name="rec")
                                    nc.vector.reciprocal(rec[:], psss[h][:])
                                    nc.vector.tensor_tensor(
                                        yT_s[:, h, jq * 512:(jq + 1) * 512],
                                        pos[h][:], rec[:], ALU.mult)

                    # ---------- phase C: output projection ----------
                    if "C" in _ablate:
                        continue
                    with tc.tile_pool(name=f"psC{b}", bufs=4,
                                      space="PSUM") as psC:
                        for t16 in range(T // 128):
                            outsb = opool.tile([128, C], F32)
                            for ntile in range(4):
                                pout = psC.tile([128, 512], F32, tag="pout")
                                for kh in range(HL):
                                    nc.tensor.matmul(
                                        pout[:],
                                        yT_s[:, kh,
                                             t16 * 128:(t16 + 1) * 128],
                                        wot_s[:, kh,
                                              ntile * 512:(ntile + 1) * 512],
                                        start=(kh == 0), stop=(kh == HL - 1))
                                eng = nc.scalar if ntile % 2 == 0 else nc.vector
                                if ntile % 2 == 0:
                                    nc.scalar.copy(
                                        outsb[:, ntile * 512:(ntile + 1) * 512],
                                        pout[:])
                                else:
                                    nc.vector.tensor_copy(
                                        outsb[:, ntile * 512:(ntile + 1) * 512],
                                        pout[:])
                            nc.sync.dma_start(
                                out_dram[b, t16 * 128:(t16 + 1) * 128, :],
                                outsb[:])
    return nc


def _host_inputs(x, wq, wk, wv, wo, q_gain, k_gain):
    """Shard + lay out the full inputs for the 8 cores."""
    bf = ml_dtypes.bfloat16
    # rope tables in [d, t] layout with rotate-half sign baked into sin
    inv_freq = ROPE_BASE ** (-np.arange(0, D, 2, dtype=np.float32) / D)
    freqs = np.arange(T, dtype=np.float32)[:, None] * inv_freq[None, :]
    cos_t = np.cos(freqs).T.astype(np.float32)      # [64, T]
    sin_t = np.sin(freqs).T.astype(np.float32)      # [64, T]
    cosf = np.concatenate([cos_t, cos_t], 0)         # [128, T]
    sinf = np.concatenate([sin_t, -sin_t], 0)        # [128, T]

    # block-diagonal causal masks for the 4 diagonal chunk offsets
    tk = np.arange(128)[:, None]
    tq = np.arange(512)[None, :]
    masks = np.stack([(tq >= tk + 128 * r) for r in range(4)], 0)
    masks = masks.transpose(1, 0, 2).astype(bf)      # [128, 4, 512]

    scale = 1.0 / np.sqrt(np.float32(D))

    xt_by_bg = []
    for bg in range(2):
        xt = np.ascontiguousarray(
            x[2 * bg:2 * bg + 2].transpose(0, 2, 1)).astype(bf)
        xt_by_bg.append(xt)

    in_maps = []
    for core in range(NCORES):
        kv = core % KV
        bg = core // KV
        wq_sh = wq[kv * HL * D:(kv + 1) * HL * D]      # [512, C]
        wk_sh = wk[kv * D:(kv + 1) * D]                # [128, C]
        wv_sh = wv[kv * D:(kv + 1) * D]
        wo_sh = wo[:, kv * HL * D:(kv + 1) * HL * D]   # [C, 512]
        gsc = (q_gain[kv * HL:(kv + 1) * HL] * k_gain[kv] * scale)
        in_maps.append({
            "xt": xt_by_bg[bg],
            "wqt": np.ascontiguousarray(wq_sh.T).astype(bf),
            "wkt": np.ascontiguousarray(wk_sh.T).astype(bf),
            "wvt": np.ascontiguousarray(wv_sh.T).astype(bf),
            "wot": np.ascontiguousarray(wo_sh.T).astype(bf),
            "cosf": cosf,
            "sinf": sinf,
            "masks": np.ascontiguousarray(masks),
            "gsc": np.broadcast_to(gsc.astype(np.float32),
                                   (128, HL)).copy(),
        })
    return in_maps


_NC_CACHE = {}


def kernel(x, wq, wk, wv, wo, q_gain, k_gain):
    if "nc" not in _NC_CACHE:
        _NC_CACHE["nc"] = build_nc()
    nc = _NC_CACHE["nc"]
    in_maps = _host_inputs(x, wq, wk, wv, wo, q_gain, k_gain)
    res = run_bass_kernel_spmd(nc, in_maps, list(range(NCORES)))
    out = np.zeros((B, T, C), dtype=np.float32)
    for bg in range(2):
        acc = res.results[bg * KV]["out"].astype(np.float32)
        for kv in range(1, KV):
            acc = acc + res.results[bg * KV + kv]["out"]
        out[2 * bg:2 * bg + 2] = acc
    return out
